# revision 1
# baseline (speedup 1.0000x reference)
"""MCAN kernel for 8 Trainium2 NeuronCores.

Strategy (per spec sharding_hint): pure data parallel over batch — B=256 is
sharded 32-per-core across the 8 cores via a shard_map'd program compiled by
neuronx-cc to a NEFF per core; all weights are replicated. The sequential
LSTM scans keep per-device work independent, so no collectives are needed.
The full program (highway -> co/self attention -> cast features -> 4 LSTM
scans -> pooling -> FC) runs on-device; the host only shards inputs and
gathers the [256, 200] output.
"""
import numpy as np
import jax
import jax.numpy as jnp
from jax.sharding import Mesh, PartitionSpec as P
from jax.experimental.shard_map import shard_map

B, LQ, LD, D, H = 256, 128, 128, 300, 200
N_CORES = 8
EPS = 1e-15

_WEIGHT_NAMES = (
    "M",
    "hwq_Wt", "hwq_bt", "hwq_Wh", "hwq_bh",
    "hwd_Wt", "hwd_bt", "hwd_Wh", "hwd_bh",
    "cast_Wc", "cast_bc", "cast_Wm", "cast_bm", "cast_Ws", "cast_bs",
    "qWx", "qWh", "qb", "dWx", "dWh", "db", "fc1_W", "fc1_b",
)


def _highway(x, Wh_, bh_, Wt_, bt_):
    h = jax.nn.relu(x @ Wh_ + bh_)
    t = jax.nn.sigmoid(x @ Wt_ + bt_)
    return t * h + (1.0 - t) * x


def _self_attention(x, key_mask):
    NEG = jnp.finfo(jnp.float32).min
    sc = jnp.einsum('bqd,bkd->bqk', x, x)
    sc = sc + (1.0 - jnp.swapaxes(key_mask, 1, 2)) * NEG
    return jax.nn.softmax(sc, axis=2) @ x


def _lstm(x, Wx, Wh, b, reverse):
    xs = jnp.swapaxes(x, 0, 1)
    if reverse:
        xs = xs[::-1]
    # hoist the input projection out of the scan: one big matmul instead of
    # 128 thin ones inside the sequential loop
    zx = xs @ Wx + b
    h0 = jnp.zeros((x.shape[0], Wh.shape[0]), x.dtype)

    def step(carry, zxt):
        h, c = carry
        z = zxt + h @ Wh
        i, f, g, o = jnp.split(z, 4, axis=-1)
        c = jax.nn.sigmoid(f) * c + jax.nn.sigmoid(i) * jnp.tanh(g)
        h = jax.nn.sigmoid(o) * jnp.tanh(c)
        return (h, c), h

    _, hs = jax.lax.scan(step, (h0, h0), zx)
    if reverse:
        hs = hs[::-1]
    return jnp.swapaxes(hs, 0, 1)


def _forward(query_emb, doc_emb, query_len, doc_len, M,
             hwq_Wt, hwq_bt, hwq_Wh, hwq_bh,
             hwd_Wt, hwd_bt, hwd_Wh, hwd_bh,
             cast_Wc, cast_bc, cast_Wm, cast_bm, cast_Ws, cast_bs,
             qWx, qWh, qb, dWx, dWh, db, fc1_W, fc1_b):
    NEG = jnp.finfo(jnp.float32).min
    q_mask = (jnp.arange(LQ) < query_len[:, None]).astype(jnp.float32)[..., None]
    d_mask = (jnp.arange(LD) < doc_len[:, None]).astype(jnp.float32)[..., None]

    qo = _highway(query_emb, hwq_Wh, hwq_bh, hwq_Wt, hwq_bt)
    do = _highway(doc_emb, hwd_Wh, hwd_bh, hwd_Wt, hwd_bt)

    S = jnp.einsum('bqd,de,bke->bqk', qo, M, do)
    S_mask = q_mask * jnp.swapaxes(d_mask, 1, 2)
    S_mean = S * S_mask
    S_am = S + (1.0 - S_mask) * NEG

    ql = query_len.astype(jnp.float32)
    dl = doc_len.astype(jnp.float32)

    q_score = jax.nn.softmax(jnp.max(S_am, axis=2, keepdims=True), axis=1)
    q_maxp = jnp.sum(q_score * qo, axis=1)
    d_score = jax.nn.softmax(jnp.max(S_am, axis=1, keepdims=True), axis=2)
    d_maxp = jnp.sum(jnp.swapaxes(d_score, 1, 2) * do, axis=1)

    q_score = jax.nn.softmax(jnp.sum(S_mean, axis=2, keepdims=True) / (dl[:, None, None] + EPS), axis=1)
    q_meanp = jnp.sum(q_score * qo, axis=1)
    d_score = jax.nn.softmax(jnp.sum(S_mean, axis=1, keepdims=True) / (ql[:, None, None] + EPS), axis=2)
    d_meanp = jnp.sum(jnp.swapaxes(d_score, 1, 2) * do, axis=1)

    q_align = jax.nn.softmax(S_am, axis=2) @ do
    d_align = jnp.einsum('bqk,bqd->bkd', jax.nn.softmax(S_am, axis=1), qo)

    q_self = _self_attention(qo, q_mask)
    d_self = _self_attention(do, d_mask)

    q_maxp_t = jnp.broadcast_to(q_maxp[:, None, :], qo.shape)
    q_meanp_t = jnp.broadcast_to(q_meanp[:, None, :], qo.shape)
    d_maxp_t = jnp.broadcast_to(d_maxp[:, None, :], do.shape)
    d_meanp_t = jnp.broadcast_to(d_meanp[:, None, :], do.shape)

    def cast(x, y, i):
        fc = jax.nn.relu(jnp.concatenate([x, y], axis=2) @ cast_Wc[i] + cast_bc[i])
        fm = jax.nn.relu((x * y) @ cast_Wm[i] + cast_bm[i])
        fs = jax.nn.relu((x - y) @ cast_Ws[i] + cast_bs[i])
        return [fc, fm, fs]

    q_feats = []
    for i, x in enumerate([q_maxp_t, q_meanp_t, q_align, q_self]):
        q_feats += cast(x, query_emb, i)
    q_cast = jnp.concatenate(q_feats + [qo], axis=2)

    d_feats = []
    for i, x in enumerate([d_maxp_t, d_meanp_t, d_align, d_self]):
        d_feats += cast(x, doc_emb, 4 + i)
    d_cast = jnp.concatenate(d_feats + [do], axis=2)

    q_hidden = jnp.concatenate([_lstm(q_cast, qWx[0], qWh[0], qb[0], False),
                                _lstm(q_cast, qWx[1], qWh[1], qb[1], True)], axis=2)
    d_hidden = jnp.concatenate([_lstm(d_cast, dWx[0], dWh[0], db[0], False),
                                _lstm(d_cast, dWx[1], dWh[1], db[1], True)], axis=2)

    q_hm = q_hidden * q_mask
    q_mean = jnp.sum(q_hm, axis=1) / (ql[:, None] + EPS)
    q_max = jnp.max(q_hm, axis=1)
    q_final = jnp.concatenate([q_mean, q_max], axis=1)

    d_hm = d_hidden * d_mask
    d_mean = jnp.sum(d_hm, axis=1) / (dl[:, None] + EPS)
    d_max = jnp.max(d_hm, axis=1)
    d_final = jnp.concatenate([d_mean, d_max], axis=1)

    final = jnp.concatenate([q_final, d_final, q_final * d_final, q_final - d_final], axis=1)
    yout = jax.nn.relu(final @ fc1_W + fc1_b)
    return yout


_COMPILED = {}


def _get_compiled():
    key = "fwd"
    if key in _COMPILED:
        return _COMPILED[key]
    devs = jax.devices()[:N_CORES]
    assert len(devs) == N_CORES, f"need {N_CORES} devices, got {len(devs)}"
    mesh = Mesh(np.asarray(devs), ("b",))
    sharded_args = ("query_emb", "doc_emb", "query_len", "doc_len")
    in_specs = tuple([P("b")] * len(sharded_args) + [P()] * len(_WEIGHT_NAMES))
    fn = jax.jit(
        shard_map(_forward, mesh=mesh, in_specs=in_specs, out_specs=P("b"),
                  check_rep=False),
        donate_argnums=(),
    )
    _COMPILED[key] = fn
    return fn


def kernel(**inputs):
    fn = _get_compiled()
    args = [np.ascontiguousarray(np.asarray(inputs[n])) for n in
            ("query_emb", "doc_emb", "query_len", "doc_len")]
    args += [np.ascontiguousarray(np.asarray(inputs[n])) for n in _WEIGHT_NAMES]
    out = fn(*args)
    return np.asarray(out).astype(np.float32)
